# revision 47
# baseline (speedup 1.0000x reference)
"""Self-contained Trainium2 Bass kernel for BoSs (block-of-states) attention.

Strategy (8 NeuronCores):
  - data-parallel over batch (2) x tensor-parallel over heads (4):
    core c handles batch c//4, q-heads [4g:4g+4] and kv-head g where g=c%4.
  - host packs tokens by state id (stable sort) so the BoSs mask becomes
    block-banded causal in packed coordinates; the k-block band per 128-wide
    q-chunk is derived from the actual segment boundaries at build time
    (39 k-blocks total vs 72 for a static band).
  - qkv and output projections run as fp8 (e4m3) hi/lo-split DoubleRow
    matmuls: W and x are split into W = Whi + Wlo (and x likewise) on the
    host; one DoubleRow pass accumulates Whi*xhi over c-chunk pairs and a
    second accumulates the cross terms (Whi*xlo + Wlo*xhi) per chunk, i.e.
    3/4 of the fp16 PE cost at ~0.1% extra error.  Weights are prescaled
    by 64 for fp8 range; the 1/64 is folded into the rope tables, the v
    copy, and the output-projection copies.  The attention core (scores,
    exp, AV) stays fp16.
  - all 4 q-heads are fused per attention matmul: scores/AV outputs are
    [128, 4, 128] (kT and vA are shared across the head group under GQA).
  - additive {0, -240} fp16 mask applied only to blocks that contain a
    segment boundary or the causal diagonal (~30 of 39); fully-valid
    blocks skip the mask (scaled scores are bounded, exp never overflows).
  - softmax denominator off the PE: Pool pre-sums P blocks and
    partition_all_reduces them; DVE takes the fp16 reciprocal and scales.
  - DMAs are few and partition-major on a single queue in priority order
    (HWDGE issue is ~625ns each and transfers serialize); hi halves lead
    lo halves so the hh pass starts while lo streams in.
  - attention and the out-projection are software-pipelined INTO the
    projection loop (waves of two projection groups, slots of
    scores/AV/out-proj between waves) to hide DMA stalls and the softmax
    latency chain.  Host sums/unpermutes the 4 TP partials.
"""

import numpy as np
from contextlib import ExitStack

# problem constants (hardcoded per spec)
B, L, HID = 2, 2048, 2048
H, KVH, D = 16, 4, 128
THETA = 10000.0
NCORES = 8
TP = 4            # tensor-parallel group size (cores per batch)
QH = H // TP      # q heads per core = 4
QC = 128          # q columns per attention chunk (= k-block width)
NJQ = L // QC     # 16
NKB = L // 128    # 16 k-blocks
NHC = HID // 128  # 16 hidden-dim chunks
LC = 512          # phase-1 L-chunk width
NLC = L // LC     # 4
MAXB = 4          # max k-blocks per q-chunk the program supports
SCALE = float(D) ** -0.5
MASK_NEG = -240.0
AV_LAG = 2        # AV runs 3 slots behind scores
P3_LAG = 5        # out-projection runs 5 slots behind scores

_CACHE = {}
LAST_EXEC_NS = None
LAST_RUN_WALL_S = None


def _seg_starts(st):
    """Packed-order segment start position for every packed position."""
    starts = {}
    out = np.empty(len(st), np.int64)
    for i, v in enumerate(st):
        if v not in starts:
            starts[v] = i
        out[i] = starts[v]
    return out


def _structure(sts):
    """Program-level band structure from the per-batch packed state ids.

    Returns (lo, masked) where lo[jq] is the first k-block of q-chunk jq and
    masked is an ordered list of (jq, kb) blocks that need an additive mask
    for at least one batch.  A block is mask-free only if it is fully valid
    (same segment, strictly causal) for EVERY batch.
    """
    los = []
    for st in sts:
        ss = _seg_starts(st)
        los.append([int(ss[jq * QC : (jq + 1) * QC].min()) // 128 for jq in range(NJQ)])
    lo = [min(l[jq] for l in los) for jq in range(NJQ)]
    masked = []
    for jq in range(NJQ):
        assert jq - lo[jq] + 1 <= MAXB, f"band {jq - lo[jq] + 1} exceeds {MAXB}"
        q0, q1 = jq * QC, (jq + 1) * QC
        for kb in range(lo[jq], jq + 1):
            k0, k1 = kb * 128, kb * 128 + 128
            need = False
            for st in sts:
                ok = (st[k0:k1, None] == st[None, q0:q1]) & (
                    np.arange(k0, k1)[:, None] <= np.arange(q0, q1)[None, :]
                )
                if not ok.all():
                    need = True
                    break
            if need:
                masked.append((jq, kb))
    return tuple(lo), tuple(masked)


def _build_nc(lo, masked):
    import concourse.tile as tile
    from concourse import bacc, bass_isa, mybir

    f32 = mybir.dt.float32
    f16 = mybir.dt.float16
    f8 = mybir.dt.float8e4
    EXP = mybir.ActivationFunctionType.Exp
    RADD = bass_isa.ReduceOp.add
    DR = mybir.MatmulPerfMode.DoubleRow

    NM = max(1, len(masked))
    mslot = {b: i for i, b in enumerate(masked)}

    nc = bacc.Bacc(
        "TRN2", target_bir_lowering=False, debug=False, num_devices=NCORES
    )

    # fp8 hi/lo split operands, laid out [p][c][t][..] with t: 0=lo, 1=hi
    # for x and t: 0=hi, 1=lo for weights, so the DoubleRow cross-term
    # matmul (Whi*xlo + Wlo*xhi) reads both tiles in natural order.
    xh = nc.dram_tensor("xh", [128, NHC * 2 * L], f8, kind="ExternalInput").ap()
    wqh = nc.dram_tensor("wqh", [128, NHC * 2 * QH * D], f8, kind="ExternalInput").ap()
    wkh = nc.dram_tensor("wkh", [128, NHC * 2 * D], f8, kind="ExternalInput").ap()
    wvh = nc.dram_tensor("wvh", [128, NHC * 2 * D], f8, kind="ExternalInput").ap()
    woh = nc.dram_tensor("woh", [128, QH * 2 * HID], f8, kind="ExternalInput").ap()
    rth = nc.dram_tensor("rth", [128, 2 * L], f16, kind="ExternalInput").ap()
    mskh = nc.dram_tensor("mskh", [128, NM * QC], f16, kind="ExternalInput").ap()
    swph = nc.dram_tensor("swph", [128, 128], f16, kind="ExternalInput").ap()
    idnh = nc.dram_tensor("idnh", [128, 128], f16, kind="ExternalInput").ap()
    out = nc.dram_tensor("out", [L, HID], f16, kind="ExternalOutput").ap()

    xh4 = xh.rearrange("p (t c l) -> p t c l", c=NHC, t=2)
    wq4 = wqh.rearrange("p (t c d) -> p t c d", c=NHC, t=2)
    wk4 = wkh.rearrange("p (t c d) -> p t c d", c=NHC, t=2)
    wv4 = wvh.rearrange("p (t c d) -> p t c d", c=NHC, t=2)

    with tile.TileContext(nc) as tc, ExitStack() as top:
        persist = top.enter_context(tc.tile_pool(name="persist", bufs=1))
        kT = persist.tile([128, L], f16, tag="kT", name="kT")
        qT = persist.tile([128, QH, L], f16, tag="qT", name="qT")
        oT8 = persist.tile([128, 2, QH, L], f8, tag="oT", name="oT8")
        vA = persist.tile([128, NKB, 128], f16, tag="vA", name="vA")
        vT_s = persist.tile([128, L], f16, tag="vT", name="vT_s")
        rt = persist.tile([128, 2, L], f16, tag="rt", name="rt")
        cosT = rt[:, 0, :]
        sinT = rt[:, 1, :]
        swp = persist.tile([128, 128], f16, tag="swp", name="swp")
        idn = persist.tile([128, 128], f16, tag="idn", name="idn")
        msk = persist.tile([128, NM, QC], f16, tag="msk", name="msk")
        wq_s = persist.tile([128, 2, NHC, QH * D], f8, tag="wq", name="wq_s")
        wk_s = persist.tile([128, 2, NHC, D], f8, tag="wk", name="wk_s")
        wv_s = persist.tile([128, 2, NHC, D], f8, tag="wv", name="wv_s")
        wo_s = persist.tile([128, 2, QH, HID], f8, tag="wo", name="wo_s")

        xpool = top.enter_context(tc.tile_pool(name="xpool", bufs=2))
        tpool = top.enter_context(tc.tile_pool(name="tpool", bufs=3))
        ppool = top.enter_context(tc.tile_pool(name="ppool", bufs=4))
        lpool = top.enter_context(tc.tile_pool(name="lpool", bufs=2))
        rpool = top.enter_context(tc.tile_pool(name="rpool", bufs=4))
        opool = top.enter_context(tc.tile_pool(name="opool", bufs=2))
        spool = top.enter_context(tc.tile_pool(name="spool", bufs=3))
        # PSUM: 8 banks. psA ([128,512] f32 = 1 bank) x3 for projections /
        # swap / phase-3; psS ([128,4,128] f32 = 1 bank) x3 score blocks;
        # psO x2 for AV accumulators (and phase-1 v transposes).
        psA = top.enter_context(tc.tile_pool(name="psA", bufs=3, space="PSUM"))
        psS = top.enter_context(tc.tile_pool(name="psS", bufs=3, space="PSUM"))
        psO = top.enter_context(tc.tile_pool(name="psO", bufs=2, space="PSUM"))

        # ---- input DMAs: ONE queue (sync) in strict priority order so the
        # HWDGE/DMA-engine serialization matches PE consumption order; hi
        # halves lead their lo counterparts (the hh pass runs first), and
        # xt1 is prefetched ahead of rope tables and masks.
        xt0 = xpool.tile([128, 2, NHC, LC], f8, tag="x", name="xt0")
        xt1 = xpool.tile([128, 2, NHC, LC], f8, tag="x", name="xt1")
        nc.sync.dma_start(xt0[:, 1, 0:8, :], xh4[:, 1, 0:8, 0:LC])
        nc.sync.dma_start(wk_s[:, 0, :, :], wk4[:, 0, :, :])
        nc.sync.dma_start(xt0[:, 1, 8:16, :], xh4[:, 1, 8:16, 0:LC])
        nc.sync.dma_start(swp[:], swph[:])
        nc.sync.dma_start(wq_s[:, 0, :, :], wq4[:, 0, :, :])
        nc.sync.dma_start(wk_s[:, 1, :, :], wk4[:, 1, :, :])
        nc.sync.dma_start(xt0[:, 0, 0:8, :], xh4[:, 0, 0:8, 0:LC])
        nc.sync.dma_start(xt0[:, 0, 8:16, :], xh4[:, 0, 8:16, 0:LC])
        nc.sync.dma_start(wq_s[:, 1, :, :], wq4[:, 1, :, :])
        nc.sync.dma_start(wv_s[:], wv4[:])
        nc.sync.dma_start(idn[:], idnh[:])
        nc.sync.dma_start(rt[:], rth.rearrange("p (t l) -> p t l", t=2))
        nc.sync.dma_start(msk[:], mskh.rearrange("p (m q) -> p m q", m=NM))
        nc.sync.dma_start(xt1[:, 1, :, :], xh4[:, 1, :, LC : 2 * LC])
        nc.sync.dma_start(xt1[:, 0, :, :], xh4[:, 0, :, LC : 2 * LC])

        pending_rope = []  # (ps, group, cols, lc) awaiting swap + rope

        def flush_rope():
            if not pending_rope:
                return
            ps, hb, cols, lc = pending_rope.pop(0)
            plain = tpool.tile([128, LC], f16, tag="plain", name=f"pl{lc}_{hb}")
            nc.scalar.copy(plain[:], ps[:])
            sw = psA.tile([128, LC], f32, tag="big", name=f"sw{lc}_{hb}")
            nc.tensor.matmul(sw[:], swp[:], plain[:], start=True, stop=True)
            t1 = tpool.tile([128, LC], f16, tag="t1", name=f"t1_{lc}_{hb}")
            nc.vector.tensor_mul(t1[:], plain[:], cosT[:, cols])
            t2 = tpool.tile([128, LC], f16, tag="t2", name=f"t2_{lc}_{hb}")
            nc.vector.tensor_mul(t2[:], sw[:], sinT[:, cols])
            dst = qT[:, hb, cols] if hb < QH else kT[:, cols]
            nc.vector.tensor_add(dst, t1[:], t2[:])

        # ---- attention + out-projection slot machinery ----
        sP = {}  # jq -> (P tile, rc tile)
        nkbs = [jq - lo[jq] + 1 for jq in range(NJQ)]

        def emit_scores(jq):
            nkb = nkbs[jq]
            qs = slice(jq * QC, (jq + 1) * QC)
            P = ppool.tile([128, MAXB, QH, QC], f16, tag="P", name=f"p{jq}")
            s_tiles = []
            for i in range(nkb):
                kb = lo[jq] + i
                s_ps = psS.tile([128, QH, QC], f32, tag="S", name=f"s{jq}_{i}")
                nc.tensor.matmul(
                    s_ps[:],
                    kT[:, kb * 128 : (kb + 1) * 128],
                    qT[:, :, qs],
                    start=True,
                    stop=True,
                )
                s_tiles.append(s_ps)
            for i in range(nkb):
                kb = lo[jq] + i
                s_ps = s_tiles[i]
                if (jq, kb) in mslot:
                    m = mslot[(jq, kb)]
                    nc.vector.tensor_add(
                        s_ps[:],
                        s_ps[:],
                        msk[:, m : m + 1, :].broadcast_to([128, QH, QC]),
                    )
                nc.scalar.activation(P[:, i, :, :], s_ps[:], EXP, scale=SCALE)
            # denominator: DVE pre-sum over blocks, Pool all-reduce partitions
            if nkb == 1:
                red_in = P[:, 0, :, :]
            else:
                pst = lpool.tile([128, QH, QC], f16, tag="pst", name=f"pst{jq}")
                nc.gpsimd.tensor_add(pst[:], P[:, 0, :, :], P[:, 1, :, :])
                for i in range(2, nkb):
                    nc.gpsimd.tensor_add(pst[:], pst[:], P[:, i, :, :])
                red_in = pst[:]
            lsum = lpool.tile([128, QH, QC], f32, tag="ls", name=f"ls{jq}")
            nc.gpsimd.partition_all_reduce(lsum[:], red_in, 128, RADD)
            rc = rpool.tile([128, QH, QC], f16, tag="rc", name=f"rc{jq}")
            with nc.allow_low_precision(
                reason="fp16 1/l scales fp16 outputs; 5e-4 rel ok"
            ):
                nc.vector.reciprocal(rc[:], lsum[:])
            sP[jq] = (P, rc)

        def emit_av(jq):
            nkb = nkbs[jq]
            qs = slice(jq * QC, (jq + 1) * QC)
            P, rc = sP.pop(jq)
            o_ps = psO.tile([128, QH, QC], f32, tag="o", name=f"o{jq}")
            for i in range(nkb):
                kb = lo[jq] + i
                nc.tensor.matmul(
                    o_ps[:],
                    vA[:, kb, :],
                    P[:, i, :, :],
                    start=(i == 0),
                    stop=(i == nkb - 1),
                )
            tmp = opool.tile([128, QH, QC], f16, tag="t", name=f"av{jq}")
            nc.vector.tensor_mul(tmp[:], o_ps[:], rc[:])
            nc.scalar.copy(oT8[:, 0, :, qs], tmp[:])
            nc.vector.tensor_sub(oT8[:, 1, :, qs], tmp[:], oT8[:, 0, :, qs])

        def emit_p3(qb):
            ob = spool.tile([128, QH, 512], f16, tag="ob", name=f"ob{qb}")
            qbs = slice(qb * 128, (qb + 1) * 128)
            for hc in range(HID // 512):
                hcs = slice(hc * 512, (hc + 1) * 512)
                fpool = psA if hc % 2 == 0 else psO
                ftag = "big" if hc % 2 == 0 else "o"
                f_ps = fpool.tile([128, 512], f32, tag=ftag, name=f"f{qb}_{hc}")
                mms = [
                    (oT8[:, 0, 2 * hp : 2 * hp + 2, qbs], wo_s[:, 1, 2 * hp : 2 * hp + 2, hcs])
                    for hp in range(QH // 2)
                ] + [(oT8[:, :, h, qbs], wo_s[:, :, h, hcs]) for h in range(QH)]
                for mi, (lw, rx) in enumerate(mms):
                    nc.tensor.matmul(
                        f_ps[:],
                        lw,
                        rx,
                        start=(mi == 0),
                        stop=(mi == len(mms) - 1),
                        perf_mode=DR,
                    )
                if hc != 2:
                    nc.scalar.mul(ob[:, hc, :], f_ps[:], 1.0 / 64.0)
                else:
                    nc.vector.tensor_scalar_mul(ob[:, hc, :], f_ps[:], 1.0 / 64.0)
                if qb >= NJQ - 2 and hc % 2 == 1:
                    nc.sync.dma_start(
                        out[qbs, (hc - 1) * 512 : (hc + 1) * 512],
                        ob[:, hc - 1 : hc + 1, :].rearrange("p h n -> p (h n)"),
                    )
            if qb < NJQ - 2:
                nc.sync.dma_start(
                    out[qb * 128 : (qb + 1) * 128, :],
                    ob.rearrange("p h n -> p (h n)"),
                )

        def slot(t):
            if 0 <= t < NJQ:
                emit_scores(t)
            if 0 <= t - AV_LAG < NJQ:
                emit_av(t - AV_LAG)
            if 0 <= t - P3_LAG < NJQ:
                emit_p3(t - P3_LAG)

        # ---- phase 1: projections with attention slots interleaved ----
        for lc in range(NLC):
            cols = slice(lc * LC, (lc + 1) * LC)
            if lc == 0:
                xt = xt0
            elif lc == 1:
                xt = xt1
            else:
                xt = xpool.tile([128, 2, NHC, LC], f8, tag="x", name=f"xt{lc}")
                nc.sync.dma_start(xt[:, 1, :, :], xh4[:, 1, :, cols])
                nc.sync.dma_start(xt[:, 0, :, :], xh4[:, 0, :, cols])
            if lc == 2:
                nc.sync.dma_start(
                    wo_s[:], woh.rearrange("p (t h n) -> p t h n", h=QH, t=2)
                )
            def group_aps(hb):
                if hb < QH:
                    return wq_s[:, :, :, hb * 128 : (hb + 1) * 128]
                return wk_s[:] if hb == QH else wv_s[:]

            def post_group(hb, ps):
                if hb <= QH:
                    pending_rope.append((ps, hb, cols, lc))
                    if len(pending_rope) > 1 or lc == NLC - 1:
                        flush_rope()
                else:  # v: transposed copy (undo the x64 weight prescale)
                    nc.scalar.mul(vT_s[:, cols], ps[:], 1.0 / 64.0)
                    for kb in range(lc * (LC // 128), (lc + 1) * (LC // 128)):
                        vt_ps = psO.tile(
                            [128, QH, QC], f32, tag="o", name=f"vt{kb}"
                        )
                        nc.tensor.matmul(
                            vt_ps[:, 0, :],
                            vT_s[:, kb * 128 : (kb + 1) * 128],
                            idn[:],
                            start=True,
                            stop=True,
                        )
                        nc.scalar.copy(vA[:, kb, :], vt_ps[:, 0, :])

            # waves of two groups: hh passes (hi operands only) run first,
            # interleaved A/B per c-half, so the PE consumes hi DMAs while
            # the lo halves stream in behind them.
            for wi, (hbA, hbB) in enumerate(((QH, 0), (1, QH + 1), (2, 3))):
                wA, wB = group_aps(hbA), group_aps(hbB)
                psa = psA.tile([128, LC], f32, tag="big", name=f"ps{lc}_{hbA}")
                psb = psA.tile([128, LC], f32, tag="big", name=f"ps{lc}_{hbB}")
                halves = [
                    ("hh", 0, 4), ("hh", 4, 8),
                    ("cr", 0, 8), ("cr", 8, 16),
                ]
                first = {id(psa): True, id(psb): True}
                for kind, i0, i1 in halves:
                    for ps, w4 in ((psa, wA), (psb, wB)):
                        if kind == "hh":
                            mm = [
                                (w4[:, 0, 2 * cp : 2 * cp + 2, :],
                                 xt[:, 1, 2 * cp : 2 * cp + 2, :])
                                for cp in range(i0, i1)
                            ]
                        else:
                            mm = [
                                (w4[:, :, c, :], xt[:, :, c, :])
                                for c in range(i0, i1)
                            ]
                        last = kind == "cr" and i1 == NHC
                        for mi, (lw, rx) in enumerate(mm):
                            nc.tensor.matmul(
                                ps[:],
                                lw,
                                rx,
                                start=first[id(ps)],
                                stop=last and mi == len(mm) - 1,
                                perf_mode=DR,
                            )
                            first[id(ps)] = False
                post_group(hbA, psa)
                post_group(hbB, psb)
                if lc >= 1:
                    t0 = 4 * (lc - 1)
                    if wi == 0:
                        slot(t0)
                        slot(t0 + 1)
                    elif wi == 1:
                        slot(t0 + 2)
                    else:
                        slot(t0 + 3)
        while pending_rope:
            flush_rope()
        for t in range(4 * (NLC - 1), NJQ + P3_LAG):
            slot(t)

    nc.compile()
    return nc


def _get_nc(lo, masked):
    key = (lo, masked)
    if key not in _CACHE:
        _CACHE[key] = _build_nc(lo, masked)
    return _CACHE[key]


def kernel(hidden_states, Wq, Wk, Wv, Wo, sid, position_ids):
    global LAST_EXEC_NS, LAST_RUN_WALL_S
    import time

    from concourse.bass_utils import run_bass_kernel_spmd

    hidden = np.asarray(hidden_states, dtype=np.float32)
    Wq = np.asarray(Wq, dtype=np.float32)
    Wk = np.asarray(Wk, dtype=np.float32)
    Wv = np.asarray(Wv, dtype=np.float32)
    Wo = np.asarray(Wo, dtype=np.float32)
    sid = np.asarray(sid)
    position_ids = np.asarray(position_ids)

    f16 = np.float16

    perms, sts = [], []
    for b in range(B):
        s = sid[b].astype(np.int64)
        perm = np.argsort(s, kind="stable")
        perms.append(perm)
        sts.append(s[perm])

    lo, masked = _structure(sts)
    nc = _get_nc(lo, masked)
    NM = max(1, len(masked))

    swp = np.zeros((128, 128), f16)
    swp[(np.arange(128) + 64) % 128, np.arange(128)] = 1.0
    idn = np.eye(128, dtype=f16)

    import ml_dtypes

    f8 = ml_dtypes.float8_e4m3

    def pmaj(a):
        """[NHC*128, cols] -> partition-major [128, NHC*cols] fp16."""
        n = a.shape[0] // 128
        return np.ascontiguousarray(
            a.reshape(n, 128, -1).transpose(1, 0, 2).reshape(128, -1).astype(f16)
        )

    def pmaj8(a, lo_first):
        """[NHC*128, cols] fp32 -> hi/lo fp8 split, partition-major
        [128, n*2*cols] with layout [p][c][t][col]; t order per lo_first."""
        a = a.astype(np.float32)
        hi = a.astype(f8)
        lo = (a - hi.astype(np.float32)).astype(f8)
        pair = (lo, hi) if lo_first else (hi, lo)
        n = a.shape[0] // 128
        cols = a.shape[1]
        st = np.stack([p.reshape(n, 128, cols) for p in pair], axis=0)
        return np.ascontiguousarray(
            st.transpose(2, 0, 1, 3).reshape(128, 2 * n * cols)
        )

    in_maps = []
    for b in range(B):
        st = sts[b]
        perm = perms[b]

        pos = position_ids[b][perm].astype(np.float32)
        inv = (
            1.0
            / (THETA ** (np.arange(0, D, 2, dtype=np.float32) / np.float32(D)))
        ).astype(np.float32)
        fr = pos[:, None] * inv[None, :]
        emb = np.concatenate([fr, fr], axis=1)  # [L, D]
        # 1/64 undoes the x64 weight prescale (fp8 range headroom)
        cosT = (np.cos(emb).T / 64.0).astype(f16)
        sinT = (np.sin(emb).T / 64.0).astype(np.float32).copy()
        sinT[: D // 2] *= -1.0  # fold rotate_half sign
        rthp = np.ascontiguousarray(
            np.concatenate([cosT, sinT.astype(f16)], axis=1)
        )

        xT = hidden[b].T[:, perm]  # [HID, L] packed
        xhp = pmaj8(xT, lo_first=True)

        mskd = np.zeros((128, NM, QC), f16)
        ki = np.arange(128)
        qi = np.arange(QC)
        for mi, (jq, kb) in enumerate(masked):
            kidx = kb * 128 + ki
            qidx = jq * QC + qi
            ok = (st[kidx][:, None] == st[qidx][None, :]) & (
                kidx[:, None] <= qidx[None, :]
            )
            mskd[:, mi, :] = np.where(ok, 0.0, MASK_NEG).astype(f16)
        mskh = np.ascontiguousarray(mskd.reshape(128, NM * QC))

        for g in range(TP):
            in_maps.append(
                dict(
                    xh=xhp,
                    wqh=pmaj8(64.0 * Wq[g * 512 : (g + 1) * 512].T, lo_first=False),
                    wkh=pmaj8(64.0 * Wk[g * 128 : (g + 1) * 128].T, lo_first=False),
                    wvh=pmaj8(64.0 * Wv[g * 128 : (g + 1) * 128].T, lo_first=False),
                    woh=pmaj8(64.0 * Wo[:, g * 512 : (g + 1) * 512].T, lo_first=True),
                    rth=rthp,
                    mskh=mskh,
                    swph=swp,
                    idnh=idn,
                )
            )

    t0 = time.time()
    res = run_bass_kernel_spmd(nc, in_maps, core_ids=list(range(NCORES)))
    LAST_RUN_WALL_S = time.time() - t0
    LAST_EXEC_NS = res.exec_time_ns

    full = np.empty((B, L, HID), np.float32)
    for b in range(B):
        acc = np.asarray(res.results[4 * b]["out"]).astype(np.float32)
        for g in range(1, TP):
            acc += np.asarray(res.results[4 * b + g]["out"]).astype(np.float32)
        unp = np.empty_like(acc)
        unp[perms[b]] = acc
        full[b] = unp
    return full


# revision 48
# speedup vs baseline: 1.0026x; 1.0026x over previous
"""Self-contained Trainium2 Bass kernel for BoSs (block-of-states) attention.

Strategy (8 NeuronCores):
  - data-parallel over batch (2) x tensor-parallel over heads (4):
    core c handles batch c//4, q-heads [4g:4g+4] and kv-head g where g=c%4.
  - host packs tokens by state id (stable sort) so the BoSs mask becomes
    block-banded causal in packed coordinates; the k-block band per 128-wide
    q-chunk is derived from the actual segment boundaries at build time
    (39 k-blocks total vs 72 for a static band).
  - qkv and output projections run as fp8 (e4m3) hi/lo-split DoubleRow
    matmuls: W and x are split into W = Whi + Wlo (and x likewise) on the
    host; one DoubleRow pass accumulates Whi*xhi over c-chunk pairs and a
    second accumulates the cross terms (Whi*xlo + Wlo*xhi) per chunk, i.e.
    3/4 of the fp16 PE cost at ~0.1% extra error.  Weights are prescaled
    by 64 for fp8 range; the 1/64 is folded into the rope tables, the v
    copy, and the output-projection copies.  The attention core (scores,
    exp, AV) stays fp16.
  - all 4 q-heads are fused per attention matmul: scores/AV outputs are
    [128, 4, 128] (kT and vA are shared across the head group under GQA).
  - additive {0, -240} fp16 mask applied only to blocks that contain a
    segment boundary or the causal diagonal (~30 of 39); fully-valid
    blocks skip the mask (scaled scores are bounded, exp never overflows).
  - softmax denominator off the PE: Pool pre-sums P blocks and
    partition_all_reduces them; DVE takes the fp16 reciprocal and scales.
  - DMAs are few and partition-major on a single queue in priority order
    (HWDGE issue is ~625ns each and transfers serialize); hi halves lead
    lo halves so the hh pass starts while lo streams in.
  - attention and the out-projection are software-pipelined INTO the
    projection loop (waves of two projection groups, slots of
    scores/AV/out-proj between waves) to hide DMA stalls and the softmax
    latency chain.  Host sums/unpermutes the 4 TP partials.
"""

import numpy as np
from contextlib import ExitStack

# problem constants (hardcoded per spec)
B, L, HID = 2, 2048, 2048
H, KVH, D = 16, 4, 128
THETA = 10000.0
NCORES = 8
TP = 4            # tensor-parallel group size (cores per batch)
QH = H // TP      # q heads per core = 4
QC = 128          # q columns per attention chunk (= k-block width)
NJQ = L // QC     # 16
NKB = L // 128    # 16 k-blocks
NHC = HID // 128  # 16 hidden-dim chunks
LC = 512          # phase-1 L-chunk width
NLC = L // LC     # 4
MAXB = 4          # max k-blocks per q-chunk the program supports
SCALE = float(D) ** -0.5
MASK_NEG = -240.0
AV_LAG = 2        # AV runs 3 slots behind scores
P3_LAG = 5        # out-projection runs 5 slots behind scores

_CACHE = {}
LAST_EXEC_NS = None
LAST_RUN_WALL_S = None


def _seg_starts(st):
    """Packed-order segment start position for every packed position."""
    starts = {}
    out = np.empty(len(st), np.int64)
    for i, v in enumerate(st):
        if v not in starts:
            starts[v] = i
        out[i] = starts[v]
    return out


def _structure(sts):
    """Program-level band structure from the per-batch packed state ids.

    Returns (lo, masked) where lo[jq] is the first k-block of q-chunk jq and
    masked is an ordered list of (jq, kb) blocks that need an additive mask
    for at least one batch.  A block is mask-free only if it is fully valid
    (same segment, strictly causal) for EVERY batch.
    """
    los = []
    for st in sts:
        ss = _seg_starts(st)
        los.append([int(ss[jq * QC : (jq + 1) * QC].min()) // 128 for jq in range(NJQ)])
    lo = [min(l[jq] for l in los) for jq in range(NJQ)]
    masked = []
    for jq in range(NJQ):
        assert jq - lo[jq] + 1 <= MAXB, f"band {jq - lo[jq] + 1} exceeds {MAXB}"
        q0, q1 = jq * QC, (jq + 1) * QC
        for kb in range(lo[jq], jq + 1):
            k0, k1 = kb * 128, kb * 128 + 128
            need = False
            for st in sts:
                ok = (st[k0:k1, None] == st[None, q0:q1]) & (
                    np.arange(k0, k1)[:, None] <= np.arange(q0, q1)[None, :]
                )
                if not ok.all():
                    need = True
                    break
            if need:
                masked.append((jq, kb))
    return tuple(lo), tuple(masked)


def _build_nc(lo, masked):
    import concourse.tile as tile
    from concourse import bacc, bass_isa, mybir

    f32 = mybir.dt.float32
    f16 = mybir.dt.float16
    f8 = mybir.dt.float8e4
    EXP = mybir.ActivationFunctionType.Exp
    RADD = bass_isa.ReduceOp.add
    DR = mybir.MatmulPerfMode.DoubleRow

    NM = max(1, len(masked))
    mslot = {b: i for i, b in enumerate(masked)}

    nc = bacc.Bacc(
        "TRN2", target_bir_lowering=False, debug=False, num_devices=NCORES
    )

    # fp8 hi/lo split operands, laid out [p][c][t][..] with t: 0=lo, 1=hi
    # for x and t: 0=hi, 1=lo for weights, so the DoubleRow cross-term
    # matmul (Whi*xlo + Wlo*xhi) reads both tiles in natural order.
    xh = nc.dram_tensor("xh", [128, NHC * 2 * L], f8, kind="ExternalInput").ap()
    wqh = nc.dram_tensor("wqh", [128, NHC * 2 * QH * D], f8, kind="ExternalInput").ap()
    wkh = nc.dram_tensor("wkh", [128, NHC * 2 * D], f8, kind="ExternalInput").ap()
    wvh = nc.dram_tensor("wvh", [128, NHC * 2 * D], f8, kind="ExternalInput").ap()
    woh = nc.dram_tensor("woh", [128, QH * 2 * HID], f8, kind="ExternalInput").ap()
    rth = nc.dram_tensor("rth", [128, 2 * L], f16, kind="ExternalInput").ap()
    mskh = nc.dram_tensor("mskh", [128, NM * QC], f16, kind="ExternalInput").ap()
    swph = nc.dram_tensor("swph", [128, 128], f16, kind="ExternalInput").ap()
    idnh = nc.dram_tensor("idnh", [128, 128], f16, kind="ExternalInput").ap()
    out = nc.dram_tensor("out", [L, HID], f16, kind="ExternalOutput").ap()

    xh4 = xh.rearrange("p (t c l) -> p t c l", c=NHC, t=2)
    wq4 = wqh.rearrange("p (t c d) -> p t c d", c=NHC, t=2)
    wk4 = wkh.rearrange("p (t c d) -> p t c d", c=NHC, t=2)
    wv4 = wvh.rearrange("p (t c d) -> p t c d", c=NHC, t=2)

    with tile.TileContext(nc) as tc, ExitStack() as top:
        persist = top.enter_context(tc.tile_pool(name="persist", bufs=1))
        kT = persist.tile([128, L], f16, tag="kT", name="kT")
        qT = persist.tile([128, QH, L], f16, tag="qT", name="qT")
        oT8 = persist.tile([128, 2, QH, L], f8, tag="oT", name="oT8")
        vA = persist.tile([128, NKB, 128], f16, tag="vA", name="vA")
        vT_s = persist.tile([128, L], f16, tag="vT", name="vT_s")
        rt = persist.tile([128, 2, L], f16, tag="rt", name="rt")
        cosT = rt[:, 0, :]
        sinT = rt[:, 1, :]
        swp = persist.tile([128, 128], f16, tag="swp", name="swp")
        idn = persist.tile([128, 128], f16, tag="idn", name="idn")
        msk = persist.tile([128, NM, QC], f16, tag="msk", name="msk")
        wq_s = persist.tile([128, 2, NHC, QH * D], f8, tag="wq", name="wq_s")
        wk_s = persist.tile([128, 2, NHC, D], f8, tag="wk", name="wk_s")
        wv_s = persist.tile([128, 2, NHC, D], f8, tag="wv", name="wv_s")
        wo_s = persist.tile([128, 2, QH, HID], f8, tag="wo", name="wo_s")

        xpool = top.enter_context(tc.tile_pool(name="xpool", bufs=2))
        tpool = top.enter_context(tc.tile_pool(name="tpool", bufs=6))
        ppool = top.enter_context(tc.tile_pool(name="ppool", bufs=4))
        lpool = top.enter_context(tc.tile_pool(name="lpool", bufs=4))
        rpool = top.enter_context(tc.tile_pool(name="rpool", bufs=4))
        opool = top.enter_context(tc.tile_pool(name="opool", bufs=3))
        spool = top.enter_context(tc.tile_pool(name="spool", bufs=3))
        # PSUM: 8 banks. psA ([128,512] f32 = 1 bank) x3 for projections /
        # swap / phase-3; psS ([128,4,128] f32 = 1 bank) x3 score blocks;
        # psO x2 for AV accumulators (and phase-1 v transposes).
        psA = top.enter_context(tc.tile_pool(name="psA", bufs=3, space="PSUM"))
        psS = top.enter_context(tc.tile_pool(name="psS", bufs=3, space="PSUM"))
        psO = top.enter_context(tc.tile_pool(name="psO", bufs=2, space="PSUM"))

        # ---- input DMAs: ONE queue (sync) in strict priority order so the
        # HWDGE/DMA-engine serialization matches PE consumption order; hi
        # halves lead their lo counterparts (the hh pass runs first), and
        # xt1 is prefetched ahead of rope tables and masks.
        xt0 = xpool.tile([128, 2, NHC, LC], f8, tag="x", name="xt0")
        xt1 = xpool.tile([128, 2, NHC, LC], f8, tag="x", name="xt1")
        nc.sync.dma_start(xt0[:, 1, 0:8, :], xh4[:, 1, 0:8, 0:LC])
        nc.sync.dma_start(wk_s[:, 0, :, :], wk4[:, 0, :, :])
        nc.sync.dma_start(xt0[:, 1, 8:16, :], xh4[:, 1, 8:16, 0:LC])
        nc.sync.dma_start(swp[:], swph[:])
        nc.sync.dma_start(wq_s[:, 0, :, :], wq4[:, 0, :, :])
        nc.sync.dma_start(wk_s[:, 1, :, :], wk4[:, 1, :, :])
        nc.sync.dma_start(xt0[:, 0, 0:8, :], xh4[:, 0, 0:8, 0:LC])
        nc.sync.dma_start(xt0[:, 0, 8:16, :], xh4[:, 0, 8:16, 0:LC])
        nc.sync.dma_start(wq_s[:, 1, :, :], wq4[:, 1, :, :])
        nc.sync.dma_start(wv_s[:], wv4[:])
        nc.sync.dma_start(idn[:], idnh[:])
        nc.sync.dma_start(rt[:], rth.rearrange("p (t l) -> p t l", t=2))
        nc.sync.dma_start(msk[:], mskh.rearrange("p (m q) -> p m q", m=NM))
        nc.sync.dma_start(xt1[:, 1, :, :], xh4[:, 1, :, LC : 2 * LC])
        nc.sync.dma_start(xt1[:, 0, :, :], xh4[:, 0, :, LC : 2 * LC])

        pending_rope = []  # (ps, group, cols, lc) awaiting swap + rope

        def flush_rope():
            if not pending_rope:
                return
            ps, hb, cols, lc = pending_rope.pop(0)
            plain = tpool.tile([128, LC], f16, tag="plain", name=f"pl{lc}_{hb}")
            nc.scalar.copy(plain[:], ps[:])
            sw = psA.tile([128, LC], f32, tag="big", name=f"sw{lc}_{hb}")
            nc.tensor.matmul(sw[:], swp[:], plain[:], start=True, stop=True)
            t1 = tpool.tile([128, LC], f16, tag="t1", name=f"t1_{lc}_{hb}")
            nc.vector.tensor_mul(t1[:], plain[:], cosT[:, cols])
            t2 = tpool.tile([128, LC], f16, tag="t2", name=f"t2_{lc}_{hb}")
            nc.vector.tensor_mul(t2[:], sw[:], sinT[:, cols])
            dst = qT[:, hb, cols] if hb < QH else kT[:, cols]
            nc.vector.tensor_add(dst, t1[:], t2[:])

        # ---- attention + out-projection slot machinery ----
        sP = {}  # jq -> (P tile, rc tile)
        nkbs = [jq - lo[jq] + 1 for jq in range(NJQ)]

        def emit_scores(jq):
            nkb = nkbs[jq]
            qs = slice(jq * QC, (jq + 1) * QC)
            P = ppool.tile([128, MAXB, QH, QC], f16, tag="P", name=f"p{jq}")
            s_tiles = []
            for i in range(nkb):
                kb = lo[jq] + i
                s_ps = psS.tile([128, QH, QC], f32, tag="S", name=f"s{jq}_{i}")
                nc.tensor.matmul(
                    s_ps[:],
                    kT[:, kb * 128 : (kb + 1) * 128],
                    qT[:, :, qs],
                    start=True,
                    stop=True,
                )
                s_tiles.append(s_ps)
            for i in range(nkb):
                kb = lo[jq] + i
                s_ps = s_tiles[i]
                if (jq, kb) in mslot:
                    m = mslot[(jq, kb)]
                    nc.vector.tensor_add(
                        s_ps[:],
                        s_ps[:],
                        msk[:, m : m + 1, :].broadcast_to([128, QH, QC]),
                    )
                nc.scalar.activation(P[:, i, :, :], s_ps[:], EXP, scale=SCALE)
            # denominator: DVE pre-sum over blocks, Pool all-reduce partitions
            if nkb == 1:
                red_in = P[:, 0, :, :]
            else:
                pst = lpool.tile([128, QH, QC], f16, tag="pst", name=f"pst{jq}")
                nc.gpsimd.tensor_add(pst[:], P[:, 0, :, :], P[:, 1, :, :])
                for i in range(2, nkb):
                    nc.gpsimd.tensor_add(pst[:], pst[:], P[:, i, :, :])
                red_in = pst[:]
            lsum = lpool.tile([128, QH, QC], f32, tag="ls", name=f"ls{jq}")
            nc.gpsimd.partition_all_reduce(lsum[:], red_in, 128, RADD)
            rc = rpool.tile([128, QH, QC], f16, tag="rc", name=f"rc{jq}")
            with nc.allow_low_precision(
                reason="fp16 1/l scales fp16 outputs; 5e-4 rel ok"
            ):
                nc.vector.reciprocal(rc[:], lsum[:])
            sP[jq] = (P, rc)

        def emit_av(jq):
            nkb = nkbs[jq]
            qs = slice(jq * QC, (jq + 1) * QC)
            P, rc = sP.pop(jq)
            o_ps = psO.tile([128, QH, QC], f32, tag="o", name=f"o{jq}")
            for i in range(nkb):
                kb = lo[jq] + i
                nc.tensor.matmul(
                    o_ps[:],
                    vA[:, kb, :],
                    P[:, i, :, :],
                    start=(i == 0),
                    stop=(i == nkb - 1),
                )
            tmp = opool.tile([128, QH, QC], f16, tag="t", name=f"av{jq}")
            nc.vector.tensor_mul(tmp[:], o_ps[:], rc[:])
            nc.scalar.copy(oT8[:, 0, :, qs], tmp[:])
            nc.vector.tensor_sub(oT8[:, 1, :, qs], tmp[:], oT8[:, 0, :, qs])

        def emit_p3(qb):
            ob = spool.tile([128, QH, 512], f16, tag="ob", name=f"ob{qb}")
            qbs = slice(qb * 128, (qb + 1) * 128)
            for hc in range(HID // 512):
                hcs = slice(hc * 512, (hc + 1) * 512)
                fpool = psA if hc % 2 == 0 else psO
                ftag = "big" if hc % 2 == 0 else "o"
                f_ps = fpool.tile([128, 512], f32, tag=ftag, name=f"f{qb}_{hc}")
                mms = [
                    (oT8[:, 0, 2 * hp : 2 * hp + 2, qbs], wo_s[:, 1, 2 * hp : 2 * hp + 2, hcs])
                    for hp in range(QH // 2)
                ] + [(oT8[:, :, h, qbs], wo_s[:, :, h, hcs]) for h in range(QH)]
                for mi, (lw, rx) in enumerate(mms):
                    nc.tensor.matmul(
                        f_ps[:],
                        lw,
                        rx,
                        start=(mi == 0),
                        stop=(mi == len(mms) - 1),
                        perf_mode=DR,
                    )
                if hc != 2:
                    nc.scalar.mul(ob[:, hc, :], f_ps[:], 1.0 / 64.0)
                else:
                    nc.vector.tensor_scalar_mul(ob[:, hc, :], f_ps[:], 1.0 / 64.0)
                if qb >= NJQ - 2 and hc % 2 == 1:
                    nc.sync.dma_start(
                        out[qbs, (hc - 1) * 512 : (hc + 1) * 512],
                        ob[:, hc - 1 : hc + 1, :].rearrange("p h n -> p (h n)"),
                    )
            if qb < NJQ - 2:
                nc.sync.dma_start(
                    out[qb * 128 : (qb + 1) * 128, :],
                    ob.rearrange("p h n -> p (h n)"),
                )

        def slot(t):
            if 0 <= t < NJQ:
                emit_scores(t)
            if 0 <= t - AV_LAG < NJQ:
                emit_av(t - AV_LAG)
            if 0 <= t - P3_LAG < NJQ:
                emit_p3(t - P3_LAG)

        # ---- phase 1: projections with attention slots interleaved ----
        for lc in range(NLC):
            cols = slice(lc * LC, (lc + 1) * LC)
            if lc == 0:
                xt = xt0
            elif lc == 1:
                xt = xt1
            else:
                xt = xpool.tile([128, 2, NHC, LC], f8, tag="x", name=f"xt{lc}")
                nc.sync.dma_start(xt[:, 1, :, :], xh4[:, 1, :, cols])
                nc.sync.dma_start(xt[:, 0, :, :], xh4[:, 0, :, cols])
            if lc == 2:
                nc.sync.dma_start(
                    wo_s[:], woh.rearrange("p (t h n) -> p t h n", h=QH, t=2)
                )
            def group_aps(hb):
                if hb < QH:
                    return wq_s[:, :, :, hb * 128 : (hb + 1) * 128]
                return wk_s[:] if hb == QH else wv_s[:]

            def post_group(hb, ps):
                if hb <= QH:
                    pending_rope.append((ps, hb, cols, lc))
                    if len(pending_rope) > 1 or lc == NLC - 1:
                        flush_rope()
                else:  # v: transposed copy (undo the x64 weight prescale)
                    nc.scalar.mul(vT_s[:, cols], ps[:], 1.0 / 64.0)
                    for kb in range(lc * (LC // 128), (lc + 1) * (LC // 128)):
                        vt_ps = psO.tile(
                            [128, QH, QC], f32, tag="o", name=f"vt{kb}"
                        )
                        nc.tensor.matmul(
                            vt_ps[:, 0, :],
                            vT_s[:, kb * 128 : (kb + 1) * 128],
                            idn[:],
                            start=True,
                            stop=True,
                        )
                        nc.scalar.copy(vA[:, kb, :], vt_ps[:, 0, :])

            # waves of two groups: hh passes (hi operands only) run first,
            # interleaved A/B per c-half, so the PE consumes hi DMAs while
            # the lo halves stream in behind them.
            for wi, (hbA, hbB) in enumerate(((QH, 0), (1, QH + 1), (2, 3))):
                wA, wB = group_aps(hbA), group_aps(hbB)
                psa = psA.tile([128, LC], f32, tag="big", name=f"ps{lc}_{hbA}")
                psb = psA.tile([128, LC], f32, tag="big", name=f"ps{lc}_{hbB}")
                halves = [
                    ("hh", 0, 4), ("hh", 4, 8),
                    ("cr", 0, 8), ("cr", 8, 16),
                ]
                first = {id(psa): True, id(psb): True}
                for kind, i0, i1 in halves:
                    for ps, w4 in ((psa, wA), (psb, wB)):
                        if kind == "hh":
                            mm = [
                                (w4[:, 0, 2 * cp : 2 * cp + 2, :],
                                 xt[:, 1, 2 * cp : 2 * cp + 2, :])
                                for cp in range(i0, i1)
                            ]
                        else:
                            mm = [
                                (w4[:, :, c, :], xt[:, :, c, :])
                                for c in range(i0, i1)
                            ]
                        last = kind == "cr" and i1 == NHC
                        for mi, (lw, rx) in enumerate(mm):
                            nc.tensor.matmul(
                                ps[:],
                                lw,
                                rx,
                                start=first[id(ps)],
                                stop=last and mi == len(mm) - 1,
                                perf_mode=DR,
                            )
                            first[id(ps)] = False
                post_group(hbA, psa)
                post_group(hbB, psb)
                if lc >= 1:
                    t0 = 4 * (lc - 1)
                    if wi == 0:
                        slot(t0)
                        slot(t0 + 1)
                    elif wi == 1:
                        slot(t0 + 2)
                    else:
                        slot(t0 + 3)
        while pending_rope:
            flush_rope()
        for t in range(4 * (NLC - 1), NJQ + P3_LAG):
            slot(t)

    nc.compile()
    return nc


def _get_nc(lo, masked):
    key = (lo, masked)
    if key not in _CACHE:
        _CACHE[key] = _build_nc(lo, masked)
    return _CACHE[key]


def kernel(hidden_states, Wq, Wk, Wv, Wo, sid, position_ids):
    global LAST_EXEC_NS, LAST_RUN_WALL_S
    import time

    from concourse.bass_utils import run_bass_kernel_spmd

    hidden = np.asarray(hidden_states, dtype=np.float32)
    Wq = np.asarray(Wq, dtype=np.float32)
    Wk = np.asarray(Wk, dtype=np.float32)
    Wv = np.asarray(Wv, dtype=np.float32)
    Wo = np.asarray(Wo, dtype=np.float32)
    sid = np.asarray(sid)
    position_ids = np.asarray(position_ids)

    f16 = np.float16

    perms, sts = [], []
    for b in range(B):
        s = sid[b].astype(np.int64)
        perm = np.argsort(s, kind="stable")
        perms.append(perm)
        sts.append(s[perm])

    lo, masked = _structure(sts)
    nc = _get_nc(lo, masked)
    NM = max(1, len(masked))

    swp = np.zeros((128, 128), f16)
    swp[(np.arange(128) + 64) % 128, np.arange(128)] = 1.0
    idn = np.eye(128, dtype=f16)

    import ml_dtypes

    f8 = ml_dtypes.float8_e4m3

    def pmaj(a):
        """[NHC*128, cols] -> partition-major [128, NHC*cols] fp16."""
        n = a.shape[0] // 128
        return np.ascontiguousarray(
            a.reshape(n, 128, -1).transpose(1, 0, 2).reshape(128, -1).astype(f16)
        )

    def pmaj8(a, lo_first):
        """[NHC*128, cols] fp32 -> hi/lo fp8 split, partition-major
        [128, n*2*cols] with layout [p][c][t][col]; t order per lo_first."""
        a = a.astype(np.float32)
        hi = a.astype(f8)
        lo = (a - hi.astype(np.float32)).astype(f8)
        pair = (lo, hi) if lo_first else (hi, lo)
        n = a.shape[0] // 128
        cols = a.shape[1]
        st = np.stack([p.reshape(n, 128, cols) for p in pair], axis=0)
        return np.ascontiguousarray(
            st.transpose(2, 0, 1, 3).reshape(128, 2 * n * cols)
        )

    in_maps = []
    for b in range(B):
        st = sts[b]
        perm = perms[b]

        pos = position_ids[b][perm].astype(np.float32)
        inv = (
            1.0
            / (THETA ** (np.arange(0, D, 2, dtype=np.float32) / np.float32(D)))
        ).astype(np.float32)
        fr = pos[:, None] * inv[None, :]
        emb = np.concatenate([fr, fr], axis=1)  # [L, D]
        # 1/64 undoes the x64 weight prescale (fp8 range headroom)
        cosT = (np.cos(emb).T / 64.0).astype(f16)
        sinT = (np.sin(emb).T / 64.0).astype(np.float32).copy()
        sinT[: D // 2] *= -1.0  # fold rotate_half sign
        rthp = np.ascontiguousarray(
            np.concatenate([cosT, sinT.astype(f16)], axis=1)
        )

        xT = hidden[b].T[:, perm]  # [HID, L] packed
        xhp = pmaj8(xT, lo_first=True)

        mskd = np.zeros((128, NM, QC), f16)
        ki = np.arange(128)
        qi = np.arange(QC)
        for mi, (jq, kb) in enumerate(masked):
            kidx = kb * 128 + ki
            qidx = jq * QC + qi
            ok = (st[kidx][:, None] == st[qidx][None, :]) & (
                kidx[:, None] <= qidx[None, :]
            )
            mskd[:, mi, :] = np.where(ok, 0.0, MASK_NEG).astype(f16)
        mskh = np.ascontiguousarray(mskd.reshape(128, NM * QC))

        for g in range(TP):
            in_maps.append(
                dict(
                    xh=xhp,
                    wqh=pmaj8(64.0 * Wq[g * 512 : (g + 1) * 512].T, lo_first=False),
                    wkh=pmaj8(64.0 * Wk[g * 128 : (g + 1) * 128].T, lo_first=False),
                    wvh=pmaj8(64.0 * Wv[g * 128 : (g + 1) * 128].T, lo_first=False),
                    woh=pmaj8(64.0 * Wo[:, g * 512 : (g + 1) * 512].T, lo_first=True),
                    rth=rthp,
                    mskh=mskh,
                    swph=swp,
                    idnh=idn,
                )
            )

    t0 = time.time()
    res = run_bass_kernel_spmd(nc, in_maps, core_ids=list(range(NCORES)))
    LAST_RUN_WALL_S = time.time() - t0
    LAST_EXEC_NS = res.exec_time_ns

    full = np.empty((B, L, HID), np.float32)
    for b in range(B):
        acc = np.asarray(res.results[4 * b]["out"]).astype(np.float32)
        for g in range(1, TP):
            acc += np.asarray(res.results[4 * b + g]["out"]).astype(np.float32)
        unp = np.empty_like(acc)
        unp[perms[b]] = acc
        full[b] = unp
    return full
